# revision 1
# baseline (speedup 1.0000x reference)
"""Trainium2 Bass kernel: batched RK4 integration of a tiny 2-4-1 LeakyReLU MLP ODE.

Math (per batch element, 99 RK4 steps, dt=1):
  dyn(s) = b2 + sum_j W2_j * lrelu(W1[0,j]*s + W1[1,j]*u + b1_j)

Folding used on device:
  y_j = s + d_j with d_j = (W1[1,j]*u + b1_j)/W1[0,j]  (per-element constant)
  W2_j*lrelu(a_j*s + c_j) = Prelu(scale_j * y_j; alpha_j) with
    W2_j >= 0: scale_j = W2_j*a_j,      alpha_j = 0.01
    W2_j <  0: scale_j = 0.01*W2_j*a_j, alpha_j = 100.0
  so k~(s) = sum_j Prelu_j(y_j) and dyn = k~ + b2.  All RK4 stage states are
  tracked as Y_j = y_j + (stage offset): the same scalar increment t_i applies
  to all four j, so one broadcast tensor add updates the state.

Sharding: pure data-parallel over batch across 8 cores (16384 elems/core laid
out as [128 partitions x 128 free]); tiny MLP params baked into the program.
"""

import sys
import os
import numpy as np

sys.path.insert(0, "/opt/trn_rl_repo")

B = 131072
T = 100
NSTEP = 99
P = 128
NCORES = 8
PER = B // NCORES          # 16384 elements per core
EF = PER // P              # 128 free columns per core

# tuning configuration
CONFIG = {
    "G": 2,            # pipelined element groups per core (divisor of EF)
    "t_dve": True,     # t_i / final scaled-copies on DVE tensor_scalar
    "dve_j": 0,        # how many of the 4 lrelu terms run on DVE (3-instr seq)
    "pool_j": 0,       # how many lrelu terms run on GPSIMD (Pool)
    "pool_final": False,  # run s_new/p-combines on Pool
    "reduce_combine": True,  # single tensor_reduce instead of pair adds
    "y_psum": False,   # Y/U tiles in PSUM (faster ACT access, slower DVE)
    "split_ys": True,  # stage-state update as two half-tiles (shorter chain)
    "chunk": 33,       # trajectory columns per output DMA
}


def _numpy_fallback(x, u, W1, b1, W2, b2):
    s = x[:, 0].astype(np.float32)
    uu = u[:, 0].astype(np.float32)
    traj = [s.copy()]
    for _ in range(NSTEP):
        def dyn(ss):
            z = np.stack([ss, uu], axis=-1)
            h = z @ W1 + b1
            h = np.where(h >= 0, h, np.float32(0.01) * h)
            return (h @ W2)[:, 0] + b2[0]
        k1 = dyn(s)
        k2 = dyn(s + np.float32(0.5) * k1)
        k3 = dyn(s + np.float32(0.5) * k2)
        k4 = dyn(s + k3)
        s = s + np.float32(1 / 6) * (k1 + 2 * k2 + 2 * k3 + k4)
        traj.append(s.copy())
    out = np.stack(traj, axis=1).astype(np.float32)
    return out[:, :, None]


def _build_program(weights, cfg=None):
    """weights = (a[4], w[4], b2) as floats; cfg overrides CONFIG."""
    from concourse import bacc, tile, mybir
    from concourse.bass_types import AP

    cfg = dict(CONFIG, **(cfg or {}))
    G = cfg["G"]
    GF = EF // G
    CHUNK = cfg["chunk"]
    a4, w4, b2 = weights
    # ACT Prelu constants (sign-folded)
    act_scale = [w * a if w >= 0 else 0.01 * w * a for a, w in zip(a4, w4)]
    act_alpha = [0.01 if w >= 0 else 100.0 for w in w4]
    # DVE/Pool lrelu constants: z = (w*a)*y; u = max(z,.01z) if w>=0 else min
    dve_m = [w * a for a, w in zip(a4, w4)]
    dve_op = ["max" if w >= 0 else "min" for w in w4]

    AF = mybir.ActivationFunctionType
    ALU = mybir.AluOpType
    f32 = mybir.dt.float32
    nc = bacc.Bacc("TRN2", target_bir_lowering=False, debug=False)

    x0 = nc.dram_tensor("x0", [P, EF], f32, kind="ExternalInput")
    yin = nc.dram_tensor("yin", [P, 4, EF], f32, kind="ExternalInput")
    out = nc.dram_tensor("out", [T, PER], f32, kind="ExternalOutput")

    n_dve_j = cfg["dve_j"]
    n_pool_j = cfg["pool_j"]
    # assignment of j-terms to engines: first ACT, then DVE, then Pool
    j_eng = ["act"] * (4 - n_dve_j - n_pool_j) + ["dve"] * n_dve_j + ["pool"] * n_pool_j

    def bcast_j(ap):
        return AP(ap.tensor, ap.offset, [ap.ap[0], [0, 4], ap.ap[1]])

    import contextlib
    with tile.TileContext(nc) as tc, contextlib.ExitStack() as stk:
        with tc.tile_pool(name="main", bufs=1) as pool:
            if cfg.get("y_psum", False):
                ypool = stk.enter_context(
                    tc.tile_pool(name="ypsum", bufs=1, space="PSUM"))
            else:
                ypool = pool
            TRJ = pool.tile([P, T * EF], f32)
            bh = pool.tile([P, 1], f32)
            bf = pool.tile([P, 1], f32)
            nc.vector.memset(bh[:], float(0.5 * b2))
            nc.vector.memset(bf[:], float(b2))

            if cfg.get("y_psum", False):
                # PSUM tiles are padded to whole 2KB banks: pack two logical
                # [P,4,GF] tensors per [P,8,GF] bank tile (3 banks per group).
                Y1, Ys, U = [], [], []
                for g in range(G):
                    b0 = ypool.tile([P, 8, GF], f32, name=f"YB0_{g}")
                    b1 = ypool.tile([P, 8, GF], f32, name=f"YB1_{g}")
                    b2t = ypool.tile([P, 8, GF], f32, name=f"YB2_{g}")
                    Y1.append([b0[:, 0:4, :], b0[:, 4:8, :]])
                    Ys.append([b1[:, 0:4, :], b1[:, 4:8, :], b2t[:, 0:4, :]])
                    U.append(b2t[:, 4:8, :])
            else:
                Y1 = [[pool.tile([P, 4, GF], f32, name=f"Y1_{g}_{i}")
                       for i in range(2)] for g in range(G)]
                Ys = [[pool.tile([P, 4, GF], f32, name=f"Ys_{g}_{i}")
                       for i in range(3)] for g in range(G)]
                U = [[pool.tile([P, 4, GF], f32, name=f"U_{g}_{i}")
                      for i in range(2)] for g in range(G)]
            Z = [pool.tile([P, 4, GF], f32, name=f"Z_{g}") for g in range(G)]
            Z2 = [pool.tile([P, 4, GF], f32, name=f"Z2_{g}") for g in range(G)]
            C = [pool.tile([P, 2, GF], f32, name=f"C_{g}") for g in range(G)]
            K = [[pool.tile([P, GF], f32, name=f"K_{g}_{i}") for i in range(4)]
                 for g in range(G)]
            TSC = [[pool.tile([P, GF], f32, name=f"T_{g}_{i}") for i in range(3)]
                   for g in range(G)]
            PP = [[pool.tile([P, GF], f32, name=f"P_{g}_{i}") for i in range(2)]
                  for g in range(G)]
            GA = [pool.tile([P, GF], f32, name=f"GA_{g}") for g in range(G)]
            GB = [pool.tile([P, GF], f32, name=f"GB_{g}") for g in range(G)]
            TT = [pool.tile([P, GF], f32, name=f"TT_{g}") for g in range(G)]

            x0raw = pool.tile([P, EF], f32)
            yinraw = pool.tile([P, 4, EF], f32)
            nc.sync.dma_start(x0raw[:], x0.ap())
            nc.sync.dma_start(yinraw[:], yin.ap())
            nc.scalar.activation(TRJ[:, 0:EF], x0raw[:], AF.Copy, bias=0.0, scale=1.0)
            for g in range(G):
                nc.scalar.activation(Y1[g][0][:], yinraw[:, :, g * GF:(g + 1) * GF],
                                     AF.Copy, bias=0.0, scale=1.0)

            qscale = [0.5, 0.5, 1.0]
            qb = [0.5 * b2, 0.5 * b2, b2]
            qbias = [bh, bh, bf]

            def emit_terms(g, ysrc, ubuf):
                for j in range(4):
                    if j_eng[j] == "act":
                        nc.scalar.activation(
                            ubuf[:, j, :], ysrc[:, j, :], AF.Prelu,
                            bias=0.0, scale=float(act_scale[j]),
                            alpha=float(act_alpha[j]))
                    else:
                        eng = nc.vector if j_eng[j] == "dve" else nc.gpsimd
                        eng.tensor_scalar(Z[g][:, j, :], ysrc[:, j, :],
                                          float(dve_m[j]), None, ALU.mult)
                        eng.tensor_scalar(Z2[g][:, j, :], Z[g][:, j, :],
                                          0.01, None, ALU.mult)
                        eng.tensor_tensor(
                            ubuf[:, j, :], Z[g][:, j, :], Z2[g][:, j, :],
                            ALU.max if dve_op[j] == "max" else ALU.min)

            for step in range(1, T):
                cur = (step - 1) % 2
                nxt = step % 2
                for stage in range(4):
                    if cfg.get("batch_terms", False):
                        for g in range(G):
                            ysrc = Y1[g][cur] if stage == 0 else Ys[g][stage - 1]
                            emit_terms(g, ysrc, U[g][stage % 2])
                    for g in range(G):
                        ysrc = Y1[g][cur] if stage == 0 else Ys[g][stage - 1]
                        if not cfg.get("batch_terms", False):
                            emit_terms(g, ysrc, U[g][stage % 2])
                        if cfg["reduce_combine"]:
                            uap = U[g][stage % 2][:]
                            u_ej = AP(uap.tensor, uap.offset,
                                      [uap.ap[0], [1, GF], [GF, 4]])
                            nc.vector.tensor_reduce(
                                K[g][stage][:], u_ej, mybir.AxisListType.X,
                                ALU.add)
                        else:
                            ub = U[g][stage % 2]
                            nc.vector.tensor_tensor(
                                C[g][:], ub[:, 0:2, :], ub[:, 2:4, :], ALU.add)
                            nc.vector.tensor_tensor(
                                K[g][stage][:], C[g][:, 0, :], C[g][:, 1, :], ALU.add)
                        if stage < 3:
                            if cfg["t_dve"]:
                                nc.vector.tensor_scalar(
                                    TSC[g][stage][:], K[g][stage][:],
                                    float(qscale[stage]), float(qb[stage]),
                                    ALU.mult, ALU.add)
                            else:
                                nc.scalar.activation(
                                    TSC[g][stage][:], K[g][stage][:], AF.Identity,
                                    bias=qbias[stage][:], scale=float(qscale[stage]))
                            if cfg.get("split_ys", False):
                                ns = cfg.get("split_n", 2)
                                w = 4 // ns
                                tsap = TSC[g][stage][:]
                                tbw = AP(tsap.tensor, tsap.offset,
                                         [tsap.ap[0], [0, w], tsap.ap[1]]) \
                                    if w > 1 else tsap
                                for h in range(ns):
                                    nc.vector.tensor_tensor(
                                        Ys[g][stage][:, h*w:(h+1)*w, :],
                                        Y1[g][cur][:, h*w:(h+1)*w, :], tbw,
                                        ALU.add)
                            else:
                                nc.vector.tensor_tensor(
                                    Ys[g][stage][:], Y1[g][cur][:],
                                    bcast_j(TSC[g][stage][:]), ALU.add)
                        if stage == 2 and cfg.get("order_opt", False):
                            # p2 = k2 + k3 and its scaled copy only need the
                            # stage-2/3 sums — emit them here so the
                            # end-of-step chain is just p1 -> ga -> T -> Y1'
                            nc.vector.tensor_tensor(PP[g][1][:], K[g][1][:],
                                                    K[g][2][:], ALU.add)
                            nc.vector.tensor_scalar(GB[g][:], PP[g][1][:],
                                                    float(1 / 3), None, ALU.mult)
                for g in range(G):
                    feng = nc.gpsimd if cfg["pool_final"] else nc.vector
                    feng.tensor_tensor(PP[g][0][:], K[g][0][:], K[g][3][:], ALU.add)
                    if not cfg.get("order_opt", False):
                        feng.tensor_tensor(PP[g][1][:], K[g][1][:], K[g][2][:],
                                           ALU.add)
                    if cfg["t_dve"]:
                        nc.vector.tensor_scalar(GA[g][:], PP[g][0][:],
                                                float(1 / 6), float(b2),
                                                ALU.mult, ALU.add)
                        if not cfg.get("order_opt", False):
                            nc.vector.tensor_scalar(GB[g][:], PP[g][1][:],
                                                    float(1 / 3), None, ALU.mult)
                    else:
                        nc.scalar.activation(GA[g][:], PP[g][0][:], AF.Identity,
                                             bias=bf[:], scale=float(1 / 6))
                        nc.scalar.activation(GB[g][:], PP[g][1][:], AF.Identity,
                                             bias=0.0, scale=float(1 / 3))
                    nc.vector.tensor_tensor(TT[g][:], GA[g][:], GB[g][:], ALU.add)
                    lo = g * GF
                    s_old = TRJ[:, (step - 1) * EF + lo:(step - 1) * EF + lo + GF]
                    s_new = TRJ[:, step * EF + lo:step * EF + lo + GF]
                    # Y1' gates the next step's stage-1 activations; the
                    # trajectory write only feeds the output DMA — emit Y1'
                    # first so the scheduler prioritizes the critical path.
                    if cfg.get("split_ys", False):
                        ns = cfg.get("split_n", 2)
                        w = 4 // ns
                        ttap = TT[g][:]
                        ttw = AP(ttap.tensor, ttap.offset,
                                 [ttap.ap[0], [0, w], ttap.ap[1]]) \
                            if w > 1 else ttap
                        for h in range(ns):
                            nc.vector.tensor_tensor(
                                Y1[g][nxt][:, h*w:(h+1)*w, :],
                                Y1[g][cur][:, h*w:(h+1)*w, :], ttw, ALU.add)
                    else:
                        nc.vector.tensor_tensor(Y1[g][nxt][:], Y1[g][cur][:],
                                                bcast_j(TT[g][:]), ALU.add)
                    feng.tensor_tensor(s_new, s_old, TT[g][:], ALU.add)

                if step % CHUNK == CHUNK - 1 or step == T - 1:
                    t1 = step + 1
                    t0 = (step // CHUNK) * CHUNK
                    if step == T - 1 and step % CHUNK != CHUNK - 1:
                        t0 = (step // CHUNK) * CHUNK
                    ntc = t1 - t0
                    trj_ap = TRJ[:]
                    src = AP(trj_ap.tensor, trj_ap.offset + t0 * EF,
                             [trj_ap.ap[0], [EF, ntc], [1, EF]])
                    out_ap = out.ap()
                    dst = AP(out_ap.tensor, out_ap.offset + t0 * PER,
                             [[EF, P], [PER, ntc], [1, EF]])
                    nc.sync.dma_start(dst, src)
    if not nc.is_finalized():
        nc.finalize()
    return nc


_PROGRAM_CACHE = {}


def kernel(x, u, W1, b1, W2, b2):
    x = np.asarray(x, dtype=np.float32)
    u = np.asarray(u, dtype=np.float32)
    W1 = np.asarray(W1, dtype=np.float32)
    b1 = np.asarray(b1, dtype=np.float32)
    W2 = np.asarray(W2, dtype=np.float32)
    b2 = np.asarray(b2, dtype=np.float32)

    a = W1[0, :]
    if x.shape != (B, 1) or np.any(np.abs(a) < 1e-6):
        return _numpy_fallback(x, u, W1, b1, W2, b2)

    from concourse import bass_utils

    key = (W1.tobytes(), b1.tobytes(), W2.tobytes(), b2.tobytes())
    nc = _PROGRAM_CACHE.get(key)
    if nc is None:
        nc = _build_program(([float(v) for v in a],
                             [float(v) for v in W2[:, 0]],
                             float(b2[0])))
        _PROGRAM_CACHE[key] = nc

    d = (W1[1, :][None, :] * u[:, 0][:, None] + b1[None, :]) / a[None, :]
    d = d.astype(np.float32)
    yfull = (x[:, 0][:, None] + d).astype(np.float32)

    in_maps = []
    for c in range(NCORES):
        sl = slice(c * PER, (c + 1) * PER)
        xc = x[sl, 0].reshape(P, EF)
        yc = yfull[sl].reshape(P, EF, 4).transpose(0, 2, 1)
        in_maps.append({"x0": np.ascontiguousarray(xc),
                        "yin": np.ascontiguousarray(yc)})

    res = bass_utils.run_bass_kernel_spmd(nc, in_maps, list(range(NCORES)))

    outf = np.empty((B, T), dtype=np.float32)
    for c in range(NCORES):
        dev = np.asarray(res.results[c]["out"]).reshape(T, PER)
        outf[c * PER:(c + 1) * PER, :] = dev.T
    return outf[:, :, None]



# revision 8
# speedup vs baseline: 1.9470x; 1.9470x over previous
"""Trainium2 Bass kernel: batched RK4 integration of a tiny 2-4-1 LeakyReLU MLP ODE.

Math (per batch element, 99 RK4 steps, dt=1):
  dyn(s) = b2 + sum_j w_j * lrelu(a_j*s + c_j),  a=W1[0,:], c_j=W1[1,j]*u+b1_j

Device formulation — j lives on the PARTITION dim (128 = 4j x 32 rows;
16384 elems/core as [32 rows x 512 cols], elem e = r*512 + c):
  y_j = s + d_j,   d_j = c_j / a_j
  State V[j*32+r, c] = sign(a_j) * y_j(e)     (fp32, exact)
  lrelu:  w_j*lrelu(a_j*y) = w_j*|a_j| * max(V_j, 0.01*V_j)
          -> U' = max(V, 0.01 V): one ACT Prelu (f32r output)
  stage inputs land directly in PSUM via PE matmuls:
      Z_i = t_i*sign(a)*k~ + Vr       (W(t_i)@U + I@Vr; Vr = f32r copy of V)
      k~ = sum_j (w_j|a_j|) U'_j      (per-j signs/factors in the weights)
      the t_i*b2*sign(a) constant folds into the Prelu's per-partition bias
  step:   KK = sign(a)*(k1+2k2+2k3+k4+6 b2)  (4 coef matmuls + ones matmul)
          V' = KK/6 + V   (DVE stt — EXACT fp32 state; Vr rounding only
                           perturbs the k-evaluations, ~1e-4 relative)
          Vr' = KK/6 + V  (same inputs, f32r output, for next step's I@Vr)
          s_t = sign(a_g)*V'_g - d_g, rows g = t%4  (single Pool tensor_tensor)
Two column groups (256 cols each — float32r needs >=256 for 1 cycle/row).
The wall-clock is chain-bound: 4 serial (Prelu -> matmul) rounds per step.
"""

import sys
import numpy as np

sys.path.insert(0, "/opt/trn_rl_repo")

B = 131072
T = 100
P = 128
NCORES = 8
PER = B // NCORES          # 16384 elements per core
RROWS = 32                 # element rows per j-block
COLS = PER // RROWS        # 512 element columns
NBLK = T // 4              # TRJ col-blocks (4 steps each)

CONFIG = {
    "G": 2,
    "chunk_blks": 5,       # col-blocks per output DMA
    "l1_dve": 0,           # cols of stage-1 lrelu on DVE (rest on ACT)
    "vr_pool": False,      # Vr' copy on Pool (tensor_scalar) instead of DVE stt
    "strack_pool": True,   # trajectory extraction on Pool (else DVE)
}

# init tensor column layout
C_V0 = 0
C_ND = COLS                         # negD
C_W = 2 * COLS                      # weight pack: Wh, Wf, W2, WI, Wb
NW = 5
C_B05 = C_W + NW * P
C_B10 = C_B05 + 1
NINIT = C_B10 + 1


def _numpy_fallback(x, u, W1, b1, W2, b2):
    s = x[:, 0].astype(np.float32)
    uu = u[:, 0].astype(np.float32)
    traj = [s.copy()]
    for _ in range(T - 1):
        def dyn(ss):
            z = np.stack([ss, uu], axis=-1)
            h = z @ W1 + b1
            h = np.where(h >= 0, h, np.float32(0.01) * h)
            return (h @ W2)[:, 0] + b2[0]
        k1 = dyn(s)
        k2 = dyn(s + np.float32(0.5) * k1)
        k3 = dyn(s + np.float32(0.5) * k2)
        k4 = dyn(s + k3)
        s = s + np.float32(1 / 6) * (k1 + 2 * k2 + 2 * k3 + k4)
        traj.append(s.copy())
    out = np.stack(traj, axis=1).astype(np.float32)
    return out[:, :, None]


def _build_program(sg, cfg=None):
    """sg: tuple of 4 signs of a_j (+1/-1) — baked into strack ops."""
    from concourse import bacc, tile, mybir
    from concourse.bass_types import AP

    cfg = dict(CONFIG, **(cfg or {}))
    G = cfg["G"]
    GW = COLS // G
    CB = cfg["chunk_blks"]

    AF = mybir.ActivationFunctionType
    ALU = mybir.AluOpType
    f32 = mybir.dt.float32
    f32r = mybir.dt.float32r
    nc = bacc.Bacc("TRN2", target_bir_lowering=False, debug=False)

    init = nc.dram_tensor("init", [P, NINIT], f32, kind="ExternalInput")
    out = nc.dram_tensor("out", [T, PER], f32, kind="ExternalOutput")

    with tile.TileContext(nc) as tc:
        with tc.tile_pool(name="main", bufs=1) as pool, \
             tc.tile_pool(name="ps", bufs=1, space="PSUM") as pp:
            INIT = pool.tile([P, NINIT], f32)
            TRJ = pool.tile([P, NBLK * COLS], f32)
            WR = pool.tile([P, NW * P], f32r)
            ONESF = pool.tile([P, GW], f32)
            ONESR = pool.tile([P, GW], f32r)
            V = [[pool.tile([P, GW], f32, name=f"V_{g}_{i}") for i in range(2)]
                 for g in range(G)]
            VRT = [[pool.tile([P, GW], f32r, name=f"VR_{g}_{i}") for i in range(2)]
                   for g in range(G)]
            U = [[pool.tile([P, GW], f32r, name=f"U_{g}_{i}") for i in range(4)]
                 for g in range(G)]
            # psum: one full bank per matmul accumulation target (start=True
            # resets at bank granularity — never share a bank between groups)
            ZBT = [[pp.tile([P, GW], f32, name=f"ZB_{g}_{i}") for i in range(3)]
                   for g in range(G)]
            KKT = [pp.tile([P, GW], f32, name=f"KK_{g}") for g in range(G)]
            ZB = [[ZBT[g][i][:] for i in range(3)] for g in range(G)]
            KK = [KKT[g][:] for g in range(G)]

            nc.sync.dma_start(INIT[:], init.ap())
            nc.scalar.activation(WR[:], INIT[:, C_W:C_W + NW * P],
                                 AF.Copy, bias=0.0, scale=1.0)
            Wh = WR[:, 0:P]
            Wf = WR[:, P:2 * P]
            W2 = WR[:, 2 * P:3 * P]
            WI = WR[:, 3 * P:4 * P]
            Wb = WR[:, 4 * P:5 * P]
            nc.vector.memset(ONESF[:], 1.0)
            nc.scalar.activation(ONESR[:], ONESF[:], AF.Copy, bias=0.0, scale=1.0)
            b05 = INIT[:, C_B05:C_B05 + 1]
            b10 = INIT[:, C_B10:C_B10 + 1]

            for g in range(G):
                v0 = INIT[:, C_V0 + g * GW:C_V0 + (g + 1) * GW]
                nc.scalar.activation(V[g][0][:], v0, AF.Copy, bias=0.0, scale=1.0)
                nc.scalar.activation(VRT[g][0][:], v0, AF.Copy, bias=0.0,
                                     scale=1.0)

            def strack(t, g, vrows):
                gg = t % 4
                p0, p1 = gg * RROWS, (gg + 1) * RROWS
                c0 = (t // 4) * COLS + g * GW
                nd = INIT[p0:p1, C_ND + g * GW:C_ND + (g + 1) * GW]
                eng = nc.gpsimd if cfg["strack_pool"] else nc.vector
                dst = TRJ[p0:p1, c0:c0 + GW]
                if sg[gg] > 0:
                    eng.tensor_tensor(dst, vrows(p0, p1), nd, ALU.add)
                else:
                    eng.tensor_tensor(dst, nd, vrows(p0, p1), ALU.subtract)

            for g in range(G):
                strack(0, g, lambda p0, p1, g=g:
                       INIT[p0:p1, C_V0 + g * GW:C_V0 + (g + 1) * GW])

            def dma_chunk(b):
                b0 = (b // CB) * CB
                nb = b - b0 + 1
                trj_ap = TRJ[:]
                src = AP(trj_ap.tensor, trj_ap.offset + b0 * COLS,
                         [trj_ap.ap[0], [COLS, nb], [1, COLS]])
                out_ap = out.ap()
                dst = AP(out_ap.tensor, out_ap.offset + b0 * 4 * PER,
                         [[PER, 4], [COLS, RROWS], [4 * PER, nb], [1, COLS]])
                nc.sync.dma_start(dst, src)

            l1d = cfg["l1_dve"]
            l1a = GW - l1d

            for t in range(1, T):
                cur, nxt = (t - 1) % 2, t % 2
                for g in range(G):
                    # stage-1 lrelu from exact V (SBUF)
                    vc = V[g][cur]
                    if l1a:
                        nc.scalar.activation(U[g][0][:, 0:l1a], vc[:, 0:l1a],
                                             AF.Prelu, bias=0.0, scale=1.0,
                                             alpha=0.01)
                    if l1d:
                        nc.vector.scalar_tensor_tensor(
                            U[g][0][:, l1a:GW], vc[:, l1a:GW], 0.01,
                            vc[:, l1a:GW], ALU.mult, ALU.max)
                for si, (wz, wk, bias, kst) in enumerate(
                        [(Wh, Wf, b05, True), (Wh, W2, b05, False),
                         (Wf, W2, b10, False)]):
                    for g in range(G):
                        # Z-bank: I@Vr first (ready early), then the U-reduce
                        nc.tensor.matmul(ZB[g][si][:], WI, VRT[g][cur][:],
                                         start=True, stop=False)
                        nc.tensor.matmul(ZB[g][si][:], wz, U[g][si][:],
                                         start=False, stop=True)
                        nc.tensor.matmul(KK[g][:], wk, U[g][si][:],
                                         start=kst, stop=False)
                    for g in range(G):
                        nc.scalar.activation(U[g][si + 1][:], ZB[g][si][:],
                                             AF.Prelu, bias=bias, scale=1.0,
                                             alpha=0.01)
                for g in range(G):
                    nc.tensor.matmul(KK[g][:], Wf, U[g][3][:],
                                     start=False, stop=False)
                    nc.tensor.matmul(KK[g][:], Wb, ONESR[:],
                                     start=False, stop=True)
                for g in range(G):
                    nc.vector.scalar_tensor_tensor(
                        V[g][nxt][:], KK[g][:], float(1.0 / 6.0),
                        V[g][cur][:], ALU.mult, ALU.add)
                    if cfg["vr_pool"]:
                        nc.gpsimd.tensor_scalar(VRT[g][nxt][:], V[g][nxt][:],
                                                1.0, None, ALU.mult)
                    else:
                        nc.vector.scalar_tensor_tensor(
                            VRT[g][nxt][:], KK[g][:], float(1.0 / 6.0),
                            V[g][cur][:], ALU.mult, ALU.add)
                for g in range(G):
                    strack(t, g, lambda p0, p1, g=g, nxt=nxt:
                           V[g][nxt][p0:p1, :])
                if t % 4 == 3:
                    b = t // 4
                    if (b + 1) % CB == 0 or b == NBLK - 1:
                        dma_chunk(b)
    if not nc.is_finalized():
        nc.finalize()
    return nc


_PROGRAM_CACHE = {}


def kernel(x, u, W1, b1, W2, b2):
    x = np.asarray(x, dtype=np.float32)
    u = np.asarray(u, dtype=np.float32)
    W1 = np.asarray(W1, dtype=np.float32)
    b1 = np.asarray(b1, dtype=np.float32)
    W2 = np.asarray(W2, dtype=np.float32)
    b2 = np.asarray(b2, dtype=np.float32)

    a = W1[0, :].astype(np.float64)
    w = W2[:, 0].astype(np.float64)
    if x.shape != (B, 1) or np.any(np.abs(a) < 1e-6):
        return _numpy_fallback(x, u, W1, b1, W2, b2)

    from concourse import bass_utils

    sg = tuple(1 if v > 0 else -1 for v in a)
    nc = _PROGRAM_CACHE.get(sg)
    if nc is None:
        nc = _build_program(sg)
        _PROGRAM_CACHE[sg] = nc

    b2f = float(b2[0])
    sga = np.array(sg, dtype=np.float64)
    wa = w * np.abs(a)                       # w_j * |a_j|

    eye = np.eye(RROWS, dtype=np.float64)
    Wf = np.zeros((P, P))
    for j in range(4):
        for jp in range(4):
            Wf[j * RROWS:(j + 1) * RROWS, jp * RROWS:(jp + 1) * RROWS] = \
                eye * (wa[j] * sga[jp])
    Wh = 0.5 * Wf
    W2m = 2.0 * Wf
    WI = np.eye(P)
    Wb = np.tile(np.repeat(6.0 * b2f * sga / P, RROWS)[None, :], (P, 1))

    c = u[:, 0].astype(np.float64)[:, None] * W1[1, :].astype(np.float64)[None, :] \
        + b1.astype(np.float64)[None, :]      # [B,4]
    d = c / a[None, :]

    mj_sign = np.repeat(sga, RROWS)           # [128]

    in_maps = []
    for core in range(NCORES):
        sl = slice(core * PER, (core + 1) * PER)
        xe = x[sl, 0].astype(np.float64)
        de = d[sl]
        v0 = (xe[:, None] + de) * sga[None, :]                  # [16384, 4]
        V0 = v0.reshape(RROWS, COLS, 4).transpose(2, 0, 1).reshape(P, COLS)
        ND = (-de).reshape(RROWS, COLS, 4).transpose(2, 0, 1).reshape(P, COLS)
        ini = np.zeros((P, NINIT), dtype=np.float32)
        ini[:, C_V0:C_V0 + COLS] = V0.astype(np.float32)
        ini[:, C_ND:C_ND + COLS] = ND.astype(np.float32)
        for k, Wm in enumerate([Wh, Wf, W2m, WI, Wb]):
            ini[:, C_W + k * P:C_W + (k + 1) * P] = Wm.astype(np.float32)
        ini[:, C_B05] = (0.5 * b2f) * mj_sign
        ini[:, C_B10] = b2f * mj_sign
        in_maps.append({"init": ini})

    res = bass_utils.run_bass_kernel_spmd(nc, in_maps, list(range(NCORES)))

    outf = np.empty((B, T), dtype=np.float32)
    for core in range(NCORES):
        dev = np.asarray(res.results[core]["out"]).reshape(T, PER)
        outf[core * PER:(core + 1) * PER, :] = dev.T
    return outf[:, :, None]


# revision 12
# speedup vs baseline: 1.9480x; 1.0005x over previous
"""Trainium2 Bass kernel: batched RK4 integration of a tiny 2-4-1 LeakyReLU MLP ODE.

Math (per batch element, 99 RK4 steps, dt=1):
  dyn(s) = b2 + sum_j w_j * lrelu(a_j*s + c_j),  a=W1[0,:], c_j=W1[1,j]*u+b1_j

Device formulation — j lives on the PARTITION dim (128 = 4j x 32 rows;
16384 elems/core as [32 rows x 512 cols], elem e = r*512 + c):
  y_j = s + d_j,   d_j = c_j / a_j
  State V[j*32+r, c] = sign(a_j) * y_j(e)     (fp32, exact)
  lrelu:  w_j*lrelu(a_j*y) = w_j*|a_j| * max(V_j, 0.01*V_j)
          -> U' = max(V, 0.01 V): one ACT Prelu (f32r output)
  stage inputs land directly in PSUM via PE matmuls:
      Z_i = t_i*sign(a)*k~ + Vr       (W(t_i)@U + I@Vr; Vr = f32r copy of V)
      k~ = sum_j (w_j|a_j|) U'_j      (per-j signs/factors in the weights)
      the t_i*b2*sign(a) constant folds into the Prelu's per-partition bias
  step:   KK = sign(a)*(k1+2k2+2k3+k4+6 b2)  (4 coef matmuls + ones matmul)
          V' = KK/6 + V   (DVE stt — EXACT fp32 state; Vr rounding only
                           perturbs the k-evaluations, ~1e-4 relative)
          Vr' = KK/6 + V  (same inputs, f32r output, for next step's I@Vr)
          s_t = sign(a_g)*V'_g - d_g, rows g = t%4  (single Pool tensor_tensor)
Two column groups (256 cols each — float32r needs >=256 for 1 cycle/row).
The wall-clock is chain-bound: 4 serial (Prelu -> matmul) rounds per step.
"""

import sys
import numpy as np

sys.path.insert(0, "/opt/trn_rl_repo")

B = 131072
T = 100
P = 128
NCORES = 8
PER = B // NCORES          # 16384 elements per core
RROWS = 32                 # element rows per j-block
COLS = PER // RROWS        # 512 element columns
NBLK = T // 4              # TRJ col-blocks (4 steps each)

CONFIG = {
    "G": 2,
    "chunk_blks": 5,       # col-blocks per output DMA
    "l1_dve": 0,           # cols of stage-1 lrelu on DVE (rest on ACT)
    "vr_pool": False,      # Vr' copy on Pool (tensor_scalar) instead of DVE stt
    "u12_pool": False,     # merge U1+U2 on Pool; one W2 matmul for both
    "strack_pool": True,   # trajectory extraction on Pool (else DVE)
}

# init tensor column layout
C_V0 = 0
C_ND = COLS                         # negD
C_W = 2 * COLS                      # weight pack: Wh, Wf, W2, WI, Wb
NW = 5
C_B05 = C_W + NW * P
C_B10 = C_B05 + 1
NINIT = C_B10 + 1


def _numpy_fallback(x, u, W1, b1, W2, b2):
    s = x[:, 0].astype(np.float32)
    uu = u[:, 0].astype(np.float32)
    traj = [s.copy()]
    for _ in range(T - 1):
        def dyn(ss):
            z = np.stack([ss, uu], axis=-1)
            h = z @ W1 + b1
            h = np.where(h >= 0, h, np.float32(0.01) * h)
            return (h @ W2)[:, 0] + b2[0]
        k1 = dyn(s)
        k2 = dyn(s + np.float32(0.5) * k1)
        k3 = dyn(s + np.float32(0.5) * k2)
        k4 = dyn(s + k3)
        s = s + np.float32(1 / 6) * (k1 + 2 * k2 + 2 * k3 + k4)
        traj.append(s.copy())
    out = np.stack(traj, axis=1).astype(np.float32)
    return out[:, :, None]


def _build_program(sg, cfg=None):
    """sg: tuple of 4 signs of a_j (+1/-1) — baked into strack ops."""
    from concourse import bacc, tile, mybir
    from concourse.bass_types import AP

    cfg = dict(CONFIG, **(cfg or {}))
    G = cfg["G"]
    GW = COLS // G
    CB = cfg["chunk_blks"]

    AF = mybir.ActivationFunctionType
    ALU = mybir.AluOpType
    f32 = mybir.dt.float32
    f32r = mybir.dt.float32r
    nc = bacc.Bacc("TRN2", target_bir_lowering=False, debug=False)

    init = nc.dram_tensor("init", [P, NINIT], f32, kind="ExternalInput")
    out = nc.dram_tensor("out", [T, PER], f32, kind="ExternalOutput")

    with tile.TileContext(nc) as tc:
        with tc.tile_pool(name="main", bufs=1) as pool, \
             tc.tile_pool(name="ps", bufs=1, space="PSUM") as pp:
            INIT = pool.tile([P, NINIT], f32)
            TRJ = pool.tile([P, NBLK * COLS], f32)
            WR = pool.tile([P, NW * P], f32r)
            ONESF = pool.tile([P, GW], f32)
            ONESR = pool.tile([P, GW], f32r)
            V = [[pool.tile([P, GW], f32, name=f"V_{g}_{i}") for i in range(2)]
                 for g in range(G)]
            VRT = [[pool.tile([P, GW], f32r, name=f"VR_{g}_{i}") for i in range(2)]
                   for g in range(G)]
            U = [[pool.tile([P, GW], f32r, name=f"U_{g}_{i}") for i in range(4)]
                 for g in range(G)]
            U12 = [pool.tile([P, GW], f32r, name=f"U12_{g}") for g in range(G)]
            # psum: one full bank per matmul accumulation target (start=True
            # resets at bank granularity — never share a bank between groups)
            ZBT = [[pp.tile([P, GW], f32, name=f"ZB_{g}_{i}") for i in range(3)]
                   for g in range(G)]
            KKT = [pp.tile([P, GW], f32, name=f"KK_{g}") for g in range(G)]
            ZB = [[ZBT[g][i][:] for i in range(3)] for g in range(G)]
            KK = [KKT[g][:] for g in range(G)]

            nc.sync.dma_start(INIT[:], init.ap())
            nc.scalar.activation(WR[:], INIT[:, C_W:C_W + NW * P],
                                 AF.Copy, bias=0.0, scale=1.0)
            Wh = WR[:, 0:P]
            Wf = WR[:, P:2 * P]
            W2 = WR[:, 2 * P:3 * P]
            WI = WR[:, 3 * P:4 * P]
            Wb = WR[:, 4 * P:5 * P]
            nc.vector.memset(ONESF[:], 1.0)
            nc.scalar.activation(ONESR[:], ONESF[:], AF.Copy, bias=0.0, scale=1.0)
            b05 = INIT[:, C_B05:C_B05 + 1]
            b10 = INIT[:, C_B10:C_B10 + 1]

            for g in range(G):
                v0 = INIT[:, C_V0 + g * GW:C_V0 + (g + 1) * GW]
                nc.scalar.activation(V[g][0][:], v0, AF.Copy, bias=0.0, scale=1.0)
                nc.scalar.activation(VRT[g][0][:], v0, AF.Copy, bias=0.0,
                                     scale=1.0)

            def strack(t, g, vrows):
                gg = t % 4
                p0, p1 = gg * RROWS, (gg + 1) * RROWS
                c0 = (t // 4) * COLS + g * GW
                nd = INIT[p0:p1, C_ND + g * GW:C_ND + (g + 1) * GW]
                eng = nc.gpsimd if cfg["strack_pool"] else nc.vector
                dst = TRJ[p0:p1, c0:c0 + GW]
                if sg[gg] > 0:
                    eng.tensor_tensor(dst, vrows(p0, p1), nd, ALU.add)
                else:
                    eng.tensor_tensor(dst, nd, vrows(p0, p1), ALU.subtract)

            for g in range(G):
                strack(0, g, lambda p0, p1, g=g:
                       INIT[p0:p1, C_V0 + g * GW:C_V0 + (g + 1) * GW])

            def dma_chunk(b):
                b0 = (b // CB) * CB
                nb = b - b0 + 1
                trj_ap = TRJ[:]
                src = AP(trj_ap.tensor, trj_ap.offset + b0 * COLS,
                         [trj_ap.ap[0], [COLS, nb], [1, COLS]])
                out_ap = out.ap()
                dst = AP(out_ap.tensor, out_ap.offset + b0 * 4 * PER,
                         [[PER, 4], [COLS, RROWS], [4 * PER, nb], [1, COLS]])
                nc.sync.dma_start(dst, src)

            l1d = cfg["l1_dve"]
            l1a = GW - l1d

            for t in range(1, T):
                cur, nxt = (t - 1) % 2, t % 2
                for g in range(G):
                    # stage-1 lrelu from exact V (SBUF)
                    vc = V[g][cur]
                    if l1a:
                        nc.scalar.activation(U[g][0][:, 0:l1a], vc[:, 0:l1a],
                                             AF.Prelu, bias=0.0, scale=1.0,
                                             alpha=0.01)
                    if l1d:
                        nc.vector.scalar_tensor_tensor(
                            U[g][0][:, l1a:GW], vc[:, l1a:GW], 0.01,
                            vc[:, l1a:GW], ALU.mult, ALU.max)
                u12 = cfg["u12_pool"]
                for g in range(G):
                    # KK group opens with the constant ones term (no U dep)
                    nc.tensor.matmul(KK[g][:], Wb, ONESR[:],
                                     start=True, stop=False)
                for si, (wz, bias) in enumerate(
                        [(Wh, b05), (Wh, b05), (Wf, b10)]):
                    for g in range(G):
                        # Z-bank: I@Vr first (ready early), then the U-reduce
                        nc.tensor.matmul(ZB[g][si][:], WI, VRT[g][cur][:],
                                         start=True, stop=False)
                        nc.tensor.matmul(ZB[g][si][:], wz, U[g][si][:],
                                         start=False, stop=True)
                        if si == 0:
                            nc.tensor.matmul(KK[g][:], Wf, U[g][0][:],
                                             start=False, stop=False)
                        elif not u12:
                            nc.tensor.matmul(KK[g][:], W2, U[g][si][:],
                                             start=False, stop=False)
                    for g in range(G):
                        nc.scalar.activation(U[g][si + 1][:], ZB[g][si][:],
                                             AF.Prelu, bias=bias, scale=1.0,
                                             alpha=0.01)
                    if si == 1 and u12:
                        for g in range(G):
                            nc.gpsimd.tensor_tensor(U12[g][:], U[g][1][:],
                                                    U[g][2][:], ALU.add)
                for g in range(G):
                    if u12:
                        nc.tensor.matmul(KK[g][:], W2, U12[g][:],
                                         start=False, stop=False)
                    nc.tensor.matmul(KK[g][:], Wf, U[g][3][:],
                                     start=False, stop=True)
                for g in range(G):
                    nc.vector.scalar_tensor_tensor(
                        V[g][nxt][:], KK[g][:], float(1.0 / 6.0),
                        V[g][cur][:], ALU.mult, ALU.add)
                    if cfg["vr_pool"]:
                        nc.gpsimd.tensor_scalar(VRT[g][nxt][:], V[g][nxt][:],
                                                1.0, None, ALU.mult)
                    else:
                        nc.vector.scalar_tensor_tensor(
                            VRT[g][nxt][:], KK[g][:], float(1.0 / 6.0),
                            V[g][cur][:], ALU.mult, ALU.add)
                for g in range(G):
                    strack(t, g, lambda p0, p1, g=g, nxt=nxt:
                           V[g][nxt][p0:p1, :])
                if t % 4 == 3:
                    b = t // 4
                    if (b + 1) % CB == 0 or b == NBLK - 1:
                        dma_chunk(b)
    if not nc.is_finalized():
        nc.finalize()
    return nc


_PROGRAM_CACHE = {}


def kernel(x, u, W1, b1, W2, b2):
    x = np.asarray(x, dtype=np.float32)
    u = np.asarray(u, dtype=np.float32)
    W1 = np.asarray(W1, dtype=np.float32)
    b1 = np.asarray(b1, dtype=np.float32)
    W2 = np.asarray(W2, dtype=np.float32)
    b2 = np.asarray(b2, dtype=np.float32)

    a = W1[0, :].astype(np.float64)
    w = W2[:, 0].astype(np.float64)
    if x.shape != (B, 1) or np.any(np.abs(a) < 1e-6):
        return _numpy_fallback(x, u, W1, b1, W2, b2)

    from concourse import bass_utils

    sg = tuple(1 if v > 0 else -1 for v in a)
    nc = _PROGRAM_CACHE.get(sg)
    if nc is None:
        nc = _build_program(sg)
        _PROGRAM_CACHE[sg] = nc

    b2f = float(b2[0])
    sga = np.array(sg, dtype=np.float64)
    wa = w * np.abs(a)                       # w_j * |a_j|

    eye = np.eye(RROWS, dtype=np.float64)
    Wf = np.zeros((P, P))
    for j in range(4):
        for jp in range(4):
            Wf[j * RROWS:(j + 1) * RROWS, jp * RROWS:(jp + 1) * RROWS] = \
                eye * (wa[j] * sga[jp])
    Wh = 0.5 * Wf
    W2m = 2.0 * Wf
    WI = np.eye(P)
    Wb = np.tile(np.repeat(6.0 * b2f * sga / P, RROWS)[None, :], (P, 1))

    c = u[:, 0].astype(np.float64)[:, None] * W1[1, :].astype(np.float64)[None, :] \
        + b1.astype(np.float64)[None, :]      # [B,4]
    d = c / a[None, :]

    mj_sign = np.repeat(sga, RROWS)           # [128]

    in_maps = []
    for core in range(NCORES):
        sl = slice(core * PER, (core + 1) * PER)
        xe = x[sl, 0].astype(np.float64)
        de = d[sl]
        v0 = (xe[:, None] + de) * sga[None, :]                  # [16384, 4]
        V0 = v0.reshape(RROWS, COLS, 4).transpose(2, 0, 1).reshape(P, COLS)
        ND = (-de).reshape(RROWS, COLS, 4).transpose(2, 0, 1).reshape(P, COLS)
        ini = np.zeros((P, NINIT), dtype=np.float32)
        ini[:, C_V0:C_V0 + COLS] = V0.astype(np.float32)
        ini[:, C_ND:C_ND + COLS] = ND.astype(np.float32)
        for k, Wm in enumerate([Wh, Wf, W2m, WI, Wb]):
            ini[:, C_W + k * P:C_W + (k + 1) * P] = Wm.astype(np.float32)
        ini[:, C_B05] = (0.5 * b2f) * mj_sign
        ini[:, C_B10] = b2f * mj_sign
        in_maps.append({"init": ini})

    res = bass_utils.run_bass_kernel_spmd(nc, in_maps, list(range(NCORES)))

    outf = np.empty((B, T), dtype=np.float32)
    for core in range(NCORES):
        dev = np.asarray(res.results[core]["out"]).reshape(T, PER)
        outf[core * PER:(core + 1) * PER, :] = dev.T
    return outf[:, :, None]


# revision 14
# speedup vs baseline: 1.9853x; 1.0192x over previous
"""Trainium2 Bass kernel: batched RK4 integration of a tiny 2-4-1 LeakyReLU MLP ODE.

Math (per batch element, 99 RK4 steps, dt=1):
  dyn(s) = b2 + sum_j w_j * lrelu(a_j*s + c_j),  a=W1[0,:], c_j=W1[1,j]*u+b1_j

Device formulation — j lives on the PARTITION dim (128 = 4j x 32 rows;
16384 elems/core as [32 rows x 512 cols], elem e = r*512 + c):
  y_j = s + d_j,   d_j = c_j / a_j
  State V[j*32+r, c] = sign(a_j) * y_j(e)     (fp32, exact)
  lrelu:  w_j*lrelu(a_j*y) = w_j*|a_j| * max(V_j, 0.01*V_j)
          -> U' = max(V, 0.01 V): one ACT Prelu (f32r output)
  stage inputs land directly in PSUM via PE matmuls:
      Z_i = t_i*sign(a)*k~ + Vr       (W(t_i)@U + I@Vr; Vr = f32r copy of V)
      k~ = sum_j (w_j|a_j|) U'_j      (per-j signs/factors in the weights)
      the t_i*b2*sign(a) constant folds into the Prelu's per-partition bias
  step:   KK = sign(a)*(k1+2k2+2k3+k4+6 b2)  (4 coef matmuls + ones matmul)
          V' = KK/6 + V   (DVE stt — EXACT fp32 state; Vr rounding only
                           perturbs the k-evaluations, ~1e-4 relative)
          Vr' = KK/6 + V  (same inputs, f32r output, for next step's I@Vr)
          s_t = sign(a_g)*V'_g - d_g, rows g = t%4  (single Pool tensor_tensor)
Two column groups (256 cols each — float32r needs >=256 for 1 cycle/row).
The wall-clock is chain-bound: 4 serial (Prelu -> matmul) rounds per step.
"""

import sys
import numpy as np

sys.path.insert(0, "/opt/trn_rl_repo")

B = 131072
T = 100
P = 128
NCORES = 8
PER = B // NCORES          # 16384 elements per core
RROWS = 32                 # element rows per j-block
COLS = PER // RROWS        # 512 element columns
NBLK = T // 4              # TRJ col-blocks (4 steps each)

CONFIG = {
    "G": 2,
    "chunk_blks": 4,       # col-blocks per output DMA
    "l1_dve": 0,           # cols of stage-1 lrelu on DVE (rest on ACT)
    "vr_pool": False,      # Vr' copy on Pool (tensor_scalar) instead of DVE stt
    "u12_pool": False,     # merge U1+U2 on Pool; one W2 matmul for both
    "strack_pool": False,  # trajectory extraction on Pool (else DVE)
}

# init tensor column layout
C_V0 = 0
C_ND = COLS                         # negD
C_W = 2 * COLS                      # weight pack: Wh, Wf, W2, WI, Wb
NW = 5
C_B05 = C_W + NW * P
C_B10 = C_B05 + 1
NINIT = C_B10 + 1


def _numpy_fallback(x, u, W1, b1, W2, b2):
    s = x[:, 0].astype(np.float32)
    uu = u[:, 0].astype(np.float32)
    traj = [s.copy()]
    for _ in range(T - 1):
        def dyn(ss):
            z = np.stack([ss, uu], axis=-1)
            h = z @ W1 + b1
            h = np.where(h >= 0, h, np.float32(0.01) * h)
            return (h @ W2)[:, 0] + b2[0]
        k1 = dyn(s)
        k2 = dyn(s + np.float32(0.5) * k1)
        k3 = dyn(s + np.float32(0.5) * k2)
        k4 = dyn(s + k3)
        s = s + np.float32(1 / 6) * (k1 + 2 * k2 + 2 * k3 + k4)
        traj.append(s.copy())
    out = np.stack(traj, axis=1).astype(np.float32)
    return out[:, :, None]


def _build_program(sg, cfg=None):
    """sg: tuple of 4 signs of a_j (+1/-1) — baked into strack ops."""
    from concourse import bacc, tile, mybir
    from concourse.bass_types import AP

    cfg = dict(CONFIG, **(cfg or {}))
    G = cfg["G"]
    GW = COLS // G
    CB = cfg["chunk_blks"]

    AF = mybir.ActivationFunctionType
    ALU = mybir.AluOpType
    f32 = mybir.dt.float32
    f32r = mybir.dt.float32r
    nc = bacc.Bacc("TRN2", target_bir_lowering=False, debug=False)

    init = nc.dram_tensor("init", [P, NINIT], f32, kind="ExternalInput")
    out = nc.dram_tensor("out", [T, PER], f32, kind="ExternalOutput")

    with tile.TileContext(nc) as tc:
        with tc.tile_pool(name="main", bufs=1) as pool, \
             tc.tile_pool(name="ps", bufs=1, space="PSUM") as pp:
            INIT = pool.tile([P, NINIT], f32)
            TRJ = pool.tile([P, NBLK * COLS], f32)
            WR = pool.tile([P, NW * P], f32r)
            ONESF = pool.tile([P, GW], f32)
            ONESR = pool.tile([P, GW], f32r)
            V = [[pool.tile([P, GW], f32, name=f"V_{g}_{i}") for i in range(2)]
                 for g in range(G)]
            VRT = [[pool.tile([P, GW], f32r, name=f"VR_{g}_{i}") for i in range(2)]
                   for g in range(G)]
            U = [[pool.tile([P, GW], f32r, name=f"U_{g}_{i}") for i in range(4)]
                 for g in range(G)]
            U12 = [pool.tile([P, GW], f32r, name=f"U12_{g}") for g in range(G)]
            # psum: one full bank per matmul accumulation target (start=True
            # resets at bank granularity — never share a bank between groups)
            ZBT = [[pp.tile([P, GW], f32, name=f"ZB_{g}_{i}") for i in range(3)]
                   for g in range(G)]
            KKT = [pp.tile([P, GW], f32, name=f"KK_{g}") for g in range(G)]
            ZB = [[ZBT[g][i][:] for i in range(3)] for g in range(G)]
            KK = [KKT[g][:] for g in range(G)]

            nc.sync.dma_start(INIT[:], init.ap())
            nc.scalar.activation(WR[:], INIT[:, C_W:C_W + NW * P],
                                 AF.Copy, bias=0.0, scale=1.0)
            Wh = WR[:, 0:P]
            Wf = WR[:, P:2 * P]
            W2 = WR[:, 2 * P:3 * P]
            WI = WR[:, 3 * P:4 * P]
            Wb = WR[:, 4 * P:5 * P]
            nc.vector.memset(ONESF[:], 1.0)
            nc.scalar.activation(ONESR[:], ONESF[:], AF.Copy, bias=0.0, scale=1.0)
            b05 = INIT[:, C_B05:C_B05 + 1]
            b10 = INIT[:, C_B10:C_B10 + 1]

            for g in range(G):
                v0 = INIT[:, C_V0 + g * GW:C_V0 + (g + 1) * GW]
                nc.scalar.activation(V[g][0][:], v0, AF.Copy, bias=0.0, scale=1.0)
                nc.scalar.activation(VRT[g][0][:], v0, AF.Copy, bias=0.0,
                                     scale=1.0)

            def strack(t, g, vrows):
                gg = t % 4
                p0, p1 = gg * RROWS, (gg + 1) * RROWS
                c0 = (t // 4) * COLS + g * GW
                nd = INIT[p0:p1, C_ND + g * GW:C_ND + (g + 1) * GW]
                eng = nc.gpsimd if cfg["strack_pool"] else nc.vector
                dst = TRJ[p0:p1, c0:c0 + GW]
                if sg[gg] > 0:
                    eng.tensor_tensor(dst, vrows(p0, p1), nd, ALU.add)
                else:
                    eng.tensor_tensor(dst, nd, vrows(p0, p1), ALU.subtract)

            for g in range(G):
                strack(0, g, lambda p0, p1, g=g:
                       INIT[p0:p1, C_V0 + g * GW:C_V0 + (g + 1) * GW])

            def dma_chunk(b):
                b0 = (b // CB) * CB
                nb = b - b0 + 1
                trj_ap = TRJ[:]
                src = AP(trj_ap.tensor, trj_ap.offset + b0 * COLS,
                         [trj_ap.ap[0], [COLS, nb], [1, COLS]])
                out_ap = out.ap()
                dst = AP(out_ap.tensor, out_ap.offset + b0 * 4 * PER,
                         [[PER, 4], [COLS, RROWS], [4 * PER, nb], [1, COLS]])
                nc.sync.dma_start(dst, src)

            l1d = cfg["l1_dve"]
            l1a = GW - l1d

            for t in range(1, T):
                cur, nxt = (t - 1) % 2, t % 2
                for g in range(G):
                    # stage-1 lrelu from exact V (SBUF)
                    vc = V[g][cur]
                    if l1a:
                        nc.scalar.activation(U[g][0][:, 0:l1a], vc[:, 0:l1a],
                                             AF.Prelu, bias=0.0, scale=1.0,
                                             alpha=0.01)
                    if l1d:
                        nc.vector.scalar_tensor_tensor(
                            U[g][0][:, l1a:GW], vc[:, l1a:GW], 0.01,
                            vc[:, l1a:GW], ALU.mult, ALU.max)
                u12 = cfg["u12_pool"]
                for g in range(G):
                    # KK group opens with the constant ones term (no U dep)
                    nc.tensor.matmul(KK[g][:], Wb, ONESR[:],
                                     start=True, stop=False)
                for si, (wz, bias) in enumerate(
                        [(Wh, b05), (Wh, b05), (Wf, b10)]):
                    for g in range(G):
                        # Z-bank: I@Vr first (ready early), then the U-reduce
                        nc.tensor.matmul(ZB[g][si][:], WI, VRT[g][cur][:],
                                         start=True, stop=False)
                        nc.tensor.matmul(ZB[g][si][:], wz, U[g][si][:],
                                         start=False, stop=True)
                        if si == 0:
                            nc.tensor.matmul(KK[g][:], Wf, U[g][0][:],
                                             start=False, stop=False)
                        elif not u12:
                            nc.tensor.matmul(KK[g][:], W2, U[g][si][:],
                                             start=False, stop=False)
                    for g in range(G):
                        nc.scalar.activation(U[g][si + 1][:], ZB[g][si][:],
                                             AF.Prelu, bias=bias, scale=1.0,
                                             alpha=0.01)
                    if si == 1 and u12:
                        for g in range(G):
                            nc.gpsimd.tensor_tensor(U12[g][:], U[g][1][:],
                                                    U[g][2][:], ALU.add)
                for g in range(G):
                    if u12:
                        nc.tensor.matmul(KK[g][:], W2, U12[g][:],
                                         start=False, stop=False)
                    nc.tensor.matmul(KK[g][:], Wf, U[g][3][:],
                                     start=False, stop=True)
                for g in range(G):
                    nc.vector.scalar_tensor_tensor(
                        V[g][nxt][:], KK[g][:], float(1.0 / 6.0),
                        V[g][cur][:], ALU.mult, ALU.add)
                    if cfg["vr_pool"]:
                        nc.gpsimd.tensor_scalar(VRT[g][nxt][:], V[g][nxt][:],
                                                1.0, None, ALU.mult)
                    else:
                        nc.vector.scalar_tensor_tensor(
                            VRT[g][nxt][:], KK[g][:], float(1.0 / 6.0),
                            V[g][cur][:], ALU.mult, ALU.add)
                for g in range(G):
                    strack(t, g, lambda p0, p1, g=g, nxt=nxt:
                           V[g][nxt][p0:p1, :])
                if t % 4 == 3:
                    b = t // 4
                    if (b + 1) % CB == 0 or b == NBLK - 1:
                        dma_chunk(b)
    if not nc.is_finalized():
        nc.finalize()
    return nc


_PROGRAM_CACHE = {}


def kernel(x, u, W1, b1, W2, b2):
    x = np.asarray(x, dtype=np.float32)
    u = np.asarray(u, dtype=np.float32)
    W1 = np.asarray(W1, dtype=np.float32)
    b1 = np.asarray(b1, dtype=np.float32)
    W2 = np.asarray(W2, dtype=np.float32)
    b2 = np.asarray(b2, dtype=np.float32)

    a = W1[0, :].astype(np.float64)
    w = W2[:, 0].astype(np.float64)
    if x.shape != (B, 1) or np.any(np.abs(a) < 1e-6):
        return _numpy_fallback(x, u, W1, b1, W2, b2)

    from concourse import bass_utils

    sg = tuple(1 if v > 0 else -1 for v in a)
    nc = _PROGRAM_CACHE.get(sg)
    if nc is None:
        nc = _build_program(sg)
        _PROGRAM_CACHE[sg] = nc

    b2f = float(b2[0])
    sga = np.array(sg, dtype=np.float64)
    wa = w * np.abs(a)                       # w_j * |a_j|

    eye = np.eye(RROWS, dtype=np.float64)
    Wf = np.zeros((P, P))
    for j in range(4):
        for jp in range(4):
            Wf[j * RROWS:(j + 1) * RROWS, jp * RROWS:(jp + 1) * RROWS] = \
                eye * (wa[j] * sga[jp])
    Wh = 0.5 * Wf
    W2m = 2.0 * Wf
    WI = np.eye(P)
    Wb = np.tile(np.repeat(6.0 * b2f * sga / P, RROWS)[None, :], (P, 1))

    c = u[:, 0].astype(np.float64)[:, None] * W1[1, :].astype(np.float64)[None, :] \
        + b1.astype(np.float64)[None, :]      # [B,4]
    d = c / a[None, :]

    mj_sign = np.repeat(sga, RROWS)           # [128]

    in_maps = []
    for core in range(NCORES):
        sl = slice(core * PER, (core + 1) * PER)
        xe = x[sl, 0].astype(np.float64)
        de = d[sl]
        v0 = (xe[:, None] + de) * sga[None, :]                  # [16384, 4]
        V0 = v0.reshape(RROWS, COLS, 4).transpose(2, 0, 1).reshape(P, COLS)
        ND = (-de).reshape(RROWS, COLS, 4).transpose(2, 0, 1).reshape(P, COLS)
        ini = np.zeros((P, NINIT), dtype=np.float32)
        ini[:, C_V0:C_V0 + COLS] = V0.astype(np.float32)
        ini[:, C_ND:C_ND + COLS] = ND.astype(np.float32)
        for k, Wm in enumerate([Wh, Wf, W2m, WI, Wb]):
            ini[:, C_W + k * P:C_W + (k + 1) * P] = Wm.astype(np.float32)
        ini[:, C_B05] = (0.5 * b2f) * mj_sign
        ini[:, C_B10] = b2f * mj_sign
        in_maps.append({"init": ini})

    res = bass_utils.run_bass_kernel_spmd(nc, in_maps, list(range(NCORES)))

    outf = np.empty((B, T), dtype=np.float32)
    for core in range(NCORES):
        dev = np.asarray(res.results[core]["out"]).reshape(T, PER)
        outf[core * PER:(core + 1) * PER, :] = dev.T
    return outf[:, :, None]
